# revision 10
# baseline (speedup 1.0000x reference)
"""Trainium2 Bass kernel for the 2D-LSTM (nn_Lstm2D) problem.

Reference computation (B=64, C=3, H=W=128, P=4 patch, NC=512 cells):
  - image is cut into a 32x32 grid of 4x4 patches, raster-scanned (1024 steps)
  - per step t=(i,j):  gates = [x_t, h_prevrow_j] @ W_ih.T + h_{t-1} @ W_hh.T + b
                       i,f,g,o = split(gates); c = sig(f)*c + sig(i)*tanh(g)
                       h = sig(o)*tanh(c)
  - output: h at every grid cell -> (B, 512, 32, 32)

Strategy (8 NeuronCores, data-parallel over batch, 8 batch elements/core):
  - the x / previous-row / bias contribution of the gates is accumulated
    directly in PSUM (two 4-bank half-row regions, ping-pong), one slot per
    step interleaved with the recurrence so the PE never idles long.  The
    bias rides along as a 49th "ones" feature of x.
  - per-step recurrence h @ W_hh.T accumulates onto the pre-filled PSUM
    (start=False), so the elementwise phase reads finished gates straight
    from PSUM: no gates-add, no bias-add, no SBUF staging of the pre part.
  - gate slots are laid out (f, i, g, o) and the matmuls issued in that
    order so sigmoid(f,i) can start ~1/2-way into the matmul burst.
  - h is produced directly in bf16 (it is both the next-step matmul operand
    and the kernel output; the host upcasts), fp32 only for the c state.
  - 2-row unrolled hardware loop: rows alternate between two h buffers, so
    there is no prev-row copy.
"""

import numpy as np
import ml_dtypes

B = 64
C = 3
H = W = 128
P = 4
NCELL = 512
IN = C * P * P          # 48
IN1 = IN + 1            # 49: x augmented with a constant-1 row (bias)
SY = SX = 32
NCORES = 8
BL = B // NCORES        # 8 batch elements per core
KC = NCELL // 128       # 4 contraction chunks for h
MC = (4 * NCELL) // 128  # 16 gate-dim chunks
HS = SX // 2            # 16 steps per half row
# gate slot order (f, g, i, o): the recurrence matmuls complete the f group
# first (sigmoid(f) + f*c start ~1/4 into the burst), then g (tanh), then i,
# then o which is only needed at the very end for h = sig(o)*tanh(c).
# torch gate chunk order is i(0-3), f(4-7), g(8-11), o(12-15).
SLOT_TO_ORIG = [4, 5, 6, 7, 8, 9, 10, 11, 0, 1, 2, 3, 12, 13, 14, 15]

BF16 = ml_dtypes.bfloat16


def _build_module():
    import concourse.bass as bass
    import concourse.bacc as bacc
    import concourse.tile as tile
    import concourse.mybir as mybir

    f32 = mybir.dt.float32
    bf16 = mybir.dt.bfloat16
    SIG = mybir.ActivationFunctionType.Sigmoid
    TANH = mybir.ActivationFunctionType.Tanh

    nc = bacc.Bacc()

    # x: per row 32 pos x 8 batch = 256 cols; padded with one zero row (row 32)
    x_d = nc.declare_dram_parameter("xt", [IN1, (SY + 1) * SX * BL], bf16,
                                    isOutput=False)
    whh_d = nc.declare_dram_parameter("whht", [128, KC * MC * 128], bf16,
                                      isOutput=False)
    wp_d = nc.declare_dram_parameter("wpt", [128, KC * MC * 128], bf16,
                                     isOutput=False)
    wx_d = nc.declare_dram_parameter("wxt", [IN1, MC * 128], bf16,
                                     isOutput=False)
    out_d = nc.declare_dram_parameter("out", [128, KC, SY * SX, BL], bf16,
                                      isOutput=True)

    with tile.TileContext(nc) as tc:
        with (
            tc.tile_pool(name="persist", bufs=1) as persist,
            tc.tile_pool(name="gates", bufs=3) as gpool,
            tc.tile_pool(name="psum", bufs=1, space="PSUM") as pspool,
        ):
            whh_sb = persist.tile([128, KC, MC, 128], bf16)
            wp_sb = persist.tile([128, KC, MC, 128], bf16)
            wx_sb = persist.tile([IN1, MC, 128], bf16)
            c_sb = persist.tile([128, KC, BL], f32)
            hbfA = persist.tile([128, KC, SX, BL], bf16)
            hbfB = persist.tile([128, KC, SX, BL], bf16)
            xA = persist.tile([IN1, SX * BL], bf16)
            xB = persist.tile([IN1, SX * BL], bf16)
            xC = persist.tile([IN1, SX * BL], bf16)
            x0 = persist.tile([IN1, HS * BL], bf16)

            # two half-row PSUM gate regions, split per gate group (f/g/i/o,
            # one 2KB bank each) so the elementwise phase's dependencies are
            # per-group and each activation can start mid-burst.
            psA = tuple(pspool.tile([128, 4, HS, BL], f32, name=f"psA{g}")
                        for g in range(4))
            psB = tuple(pspool.tile([128, 4, HS, BL], f32, name=f"psB{g}")
                        for g in range(4))

            def slot_view(ps, s):
                """(tile, local slot) for global gate slot s."""
                return ps[s // 4], s % 4

            nc.sync.dma_start(out=whh_sb[:], in_=whh_d[:])
            nc.sync.dma_start(out=wp_sb[:], in_=wp_d[:])
            nc.sync.dma_start(out=wx_sb[:], in_=wx_d[:])
            nc.vector.memset(c_sb[:], 0.0)
            nc.vector.memset(hbfA[:], 0.0)
            nc.vector.memset(hbfB[:], 0.0)

            # pull the sigmoid/tanh ACT table load out of the loop
            warm = persist.tile([1, 1], f32)
            nc.vector.memset(warm[:], 0.0)
            nc.scalar.activation(out=warm[:], in_=warm[:], func=SIG)
            nc.scalar.activation(out=warm[:], in_=warm[:], func=TANH)

            # start=True clears the has_written bits for the WHOLE bank, so it
            # may only be issued on the first slot of each 4-slot bank; later
            # slots overwrite (bit cleared) then accumulate (bit set).
            def prefill(ps, s, xrow, xh, hprev, hh):
                """Accumulate slot s of half-row region ps with the x/bias and
                prev-row contributions: x half xh (0/1) of xrow, h half hh of
                hprev."""
                t, ls = slot_view(ps, s)
                nc.tensor.matmul(
                    t[:, ls, :, :], wx_sb[:, s, :],
                    xrow[:, xh * HS * BL:(xh + 1) * HS * BL],
                    start=(s % 4 == 0), stop=False)
                for k in range(KC):
                    nc.tensor.matmul(
                        t[:, ls, :, :], wp_sb[:, k, s, :],
                        hprev[:, k, hh * HS:(hh + 1) * HS, :],
                        start=False, stop=False)

            # bootstrap: pre-fill region A with row 0 first half (prev row is
            # all zeros, so only the x/bias part).
            nc.sync.dma_start(out=x0[:], in_=x_d[:, 0:HS * BL])
            for s in range(MC):
                t, ls = slot_view(psA, s)
                nc.tensor.matmul(t[:, ls, :, :], wx_sb[:, s, :], x0[:],
                                 start=(s % 4 == 0), stop=False)

            def step(j, cur, prev, ps, scol, pre):
                """One LSTM step: recurrence matmuls accumulate onto the four
                gate-group PSUM tiles at column scol, then the elementwise
                phase writes h (bf16) into cur[:, :, j, :].  pre() queues the
                next half-row's pre-fill matmuls behind a nop that depends on
                this step's sigmoid(f) output, pinning them into this step's
                tensor-engine idle gap (the scheduler otherwise piles them up
                at the half-row boundary)."""
                for s in range(MC):
                    t, ls = slot_view(ps, s)
                    for k in range(KC):
                        rhs = (prev[:, k, SX - 1, :] if j == 0
                               else cur[:, k, j - 1, :])
                        nc.tensor.matmul(
                            t[:, ls, scol, :], whh_sb[:, k, s, :], rhs,
                            start=False, stop=(k == KC - 1))

                sf = gpool.tile([128, KC, BL], f32)
                tg = gpool.tile([128, KC, BL], f32)
                si = gpool.tile([128, KC, BL], f32)
                so = gpool.tile([128, KC, BL], f32)
                tc_t = gpool.tile([128, KC, BL], f32)
                fc = gpool.tile([128, KC, BL], f32)
                ig = gpool.tile([128, KC, BL], f32)
                nc.scalar.activation(out=sf[:], in_=ps[0][:, :, scol, :],
                                     func=SIG)
                nc.scalar.activation(out=tg[:], in_=ps[1][:, :, scol, :],
                                     func=TANH)
                nc.scalar.activation(out=si[:], in_=ps[2][:, :, scol, :],
                                     func=SIG)
                nc.vector.tensor_mul(fc[:], sf[:], c_sb[:])
                nc.vector.tensor_mul(ig[:], si[:], tg[:])
                nc.vector.tensor_add(c_sb[:], fc[:], ig[:])
                nc.scalar.activation(out=so[:], in_=ps[3][:, :, scol, :],
                                     func=SIG)
                nc.scalar.activation(out=tc_t[:], in_=c_sb[:], func=TANH)
                nc.vector.tensor_mul(cur[:, :, j, :], so[:], tc_t[:])

                # anchor the pre-fill for the other region into this step's
                # tensor-engine gap: a nop reading sf cannot issue before this
                # step's f-matmuls and sigmoid(f) are done.
                dep = nc.tensor.nop(hint="dep").ins
                dep.ins = [nc.tensor.lower_ap(sf[:])]
                pre()

            with tc.For_i(0, SY // 2) as iv:
                # x rows 2iv, 2iv+1, 2iv+2 (row 32 is zero padding)
                nc.gpsimd.dma_start(out=xA[:], in_=x_d[:, bass.ds(iv * 2 * SX * BL, SX * BL)])
                nc.gpsimd.dma_start(out=xB[:], in_=x_d[:, bass.ds((iv * 2 + 1) * SX * BL, SX * BL)])
                nc.gpsimd.dma_start(out=xC[:], in_=x_d[:, bass.ds((iv * 2 + 2) * SX * BL, SX * BL)])

                # ---- row 2iv: cur=hbfA, prev=hbfB
                for j in range(SX):
                    ps, scol = (psA, j) if j < HS else (psB, j - HS)
                    if j < HS:
                        # pre-fill psB slot j for this row's 2nd half
                        pre = lambda s=j: prefill(psB, s, xA, 1, hbfB, 1)
                    else:
                        # pre-fill psA slot j-16 for row 2iv+1 first half
                        pre = lambda s=j - HS: prefill(psA, s, xB, 0, hbfA, 0)
                    step(j, hbfA, hbfB, ps, scol, pre)
                nc.gpsimd.dma_start(
                    out=out_d[:, :, bass.ds(iv * 2 * SX, SX), :], in_=hbfA[:])

                # ---- row 2iv+1: cur=hbfB, prev=hbfA
                for j in range(SX):
                    ps, scol = (psA, j) if j < HS else (psB, j - HS)
                    if j < HS:
                        pre = lambda s=j: prefill(psB, s, xB, 1, hbfA, 1)
                    else:
                        # pre-fill psA for row 2iv+2 first half (x row 2iv+2;
                        # zero padding row at iv=15 - consumed never)
                        pre = lambda s=j - HS: prefill(psA, s, xC, 0, hbfB, 0)
                    step(j, hbfB, hbfA, ps, scol, pre)
                nc.gpsimd.dma_start(
                    out=out_d[:, :, bass.ds((iv * 2 + 1) * SX, SX), :],
                    in_=hbfB[:])

    nc.compile()
    return nc


_CACHE = {}


def _get_module():
    if "m" not in _CACHE:
        _CACHE["m"] = _build_module()
    return _CACHE["m"]


def _prep_shared(W_ih, W_hh, b_ih, b_hh):
    perm = np.array(SLOT_TO_ORIG)
    wih_t = np.ascontiguousarray(W_ih.T.astype(np.float32))     # (560, 2048)
    bias = (b_ih + b_hh).astype(np.float32).reshape(MC, 128)[perm]
    wx = wih_t[:IN]                                             # (48, 2048)
    wx = wx.reshape(IN, MC, 128)[:, perm, :]
    wx = np.concatenate([wx, bias[None, :, :]], axis=0)         # (49, 16, 128)
    wx = wx.reshape(IN1, MC * 128)
    wp = wih_t[IN:]                                             # (512, 2048)
    wp = wp.reshape(KC, 128, MC, 128)[:, :, perm, :]
    wp = wp.transpose(1, 0, 2, 3).reshape(128, KC * MC * 128)
    whh = np.ascontiguousarray(W_hh.T.astype(np.float32))       # (512, 2048)
    whh = whh.reshape(KC, 128, MC, 128)[:, :, perm, :]
    whh = whh.transpose(1, 0, 2, 3).reshape(128, KC * MC * 128)
    return (wx.astype(BF16), wp.astype(BF16), whh.astype(BF16))


def _prep_x(batch):
    # xs[i, j, b, :] = patch (C,P,P) flattened, matching the reference
    xs = batch.reshape(B, C, SY, P, SX, P).transpose(2, 4, 0, 1, 3, 5)
    xs = xs.reshape(SY, SX, B, IN)
    per_core = []
    for c in range(NCORES):
        xc = xs[:, :, c * BL:(c + 1) * BL, :]          # (SY, SX, BL, IN)
        xc = xc.transpose(3, 0, 1, 2).reshape(IN, SY, SX * BL)
        xc = np.concatenate(
            [xc, np.ones((1, SY, SX * BL), np.float32)], axis=0)
        xc = np.concatenate(
            [xc, np.zeros((IN1, 1, SX * BL), np.float32)], axis=1)
        per_core.append(
            np.ascontiguousarray(xc.reshape(IN1, (SY + 1) * SX * BL))
            .astype(BF16))
    return per_core


def _run(batch, W_ih, W_hh, b_ih, b_hh, trace=False):
    from concourse.bass_utils import run_bass_kernel_spmd

    batch = np.asarray(batch, dtype=np.float32)
    wx, wp, whh = _prep_shared(
        np.asarray(W_ih), np.asarray(W_hh), np.asarray(b_ih), np.asarray(b_hh))
    xs = _prep_x(batch)

    nc = _get_module()
    in_maps = [
        {"xt": xs[c], "whht": whh, "wpt": wp, "wxt": wx}
        for c in range(NCORES)
    ]
    res = run_bass_kernel_spmd(nc, in_maps, list(range(NCORES)), trace=trace)

    outs = []
    for c in range(NCORES):
        arr = np.asarray(res.results[c]["out"]).astype(np.float32)
        # arr axes (128, KC, T, BL): reference's to_image is a raw reshape of
        # (B, T, NC) into (B, NC, SY, SX): flatten (BL, T, KC*128)->(BL, T*NC).
        arr = arr.transpose(3, 2, 1, 0).reshape(BL, NCELL, SY, SX)
        outs.append(arr)
    return np.concatenate(outs, axis=0), res


def kernel(batch, W_ih, W_hh, b_ih, b_hh):
    out, _ = _run(batch, W_ih, W_hh, b_ih, b_hh)
    return out


# revision 15
# speedup vs baseline: 1.0002x; 1.0002x over previous
"""Trainium2 Bass kernel for the 2D-LSTM (nn_Lstm2D) problem.

Reference computation (B=64, C=3, H=W=128, P=4 patch, NC=512 cells):
  - image is cut into a 32x32 grid of 4x4 patches, raster-scanned (1024 steps)
  - per step t=(i,j):  gates = [x_t, h_prevrow_j] @ W_ih.T + h_{t-1} @ W_hh.T + b
                       i,f,g,o = split(gates); c = sig(f)*c + sig(i)*tanh(g)
                       h = sig(o)*tanh(c)
  - output: h at every grid cell -> (B, 512, 32, 32)

Strategy (8 NeuronCores, data-parallel over batch, 8 batch elements/core):
  - the x / previous-row / bias contribution of the gates is accumulated
    directly in PSUM (two 4-bank half-row regions, ping-pong), one slot per
    step interleaved with the recurrence so the PE never idles long.  The
    bias rides along as a 49th "ones" feature of x.
  - per-step recurrence h @ W_hh.T accumulates onto the pre-filled PSUM
    (start=False), so the elementwise phase reads finished gates straight
    from PSUM: no gates-add, no bias-add, no SBUF staging of the pre part.
  - gate slots are laid out (f, i, g, o) and the matmuls issued in that
    order so sigmoid(f,i) can start ~1/2-way into the matmul burst.
  - h is produced directly in bf16 (it is both the next-step matmul operand
    and the kernel output; the host upcasts), fp32 only for the c state.
  - 2-row unrolled hardware loop: rows alternate between two h buffers, so
    there is no prev-row copy.
"""

import numpy as np
import ml_dtypes

B = 64
C = 3
H = W = 128
P = 4
NCELL = 512
IN = C * P * P          # 48
IN1 = IN + 1            # 49: x augmented with a constant-1 row (bias)
SY = SX = 32
NCORES = 8
BL = B // NCORES        # 8 batch elements per core
KC = NCELL // 128       # 4 contraction chunks for h
MC = (4 * NCELL) // 128  # 16 gate-dim chunks
HS = SX // 2            # 16 steps per half row
# gate slot order (f, g, i, o): the recurrence matmuls complete the f group
# first (sigmoid(f) + f*c start ~1/4 into the burst), then g (tanh), then i,
# then o which is only needed at the very end for h = sig(o)*tanh(c).
# torch gate chunk order is i(0-3), f(4-7), g(8-11), o(12-15).
SLOT_TO_ORIG = [4, 5, 6, 7, 8, 9, 10, 11, 0, 1, 2, 3, 12, 13, 14, 15]

BF16 = ml_dtypes.bfloat16


def _build_module():
    import concourse.bass as bass
    import concourse.bacc as bacc
    import concourse.tile as tile
    import concourse.mybir as mybir

    f32 = mybir.dt.float32
    bf16 = mybir.dt.bfloat16
    SIG = mybir.ActivationFunctionType.Sigmoid
    TANH = mybir.ActivationFunctionType.Tanh

    nc = bacc.Bacc()

    # x: per row 32 pos x 8 batch = 256 cols; padded with one zero row (row 32)
    x_d = nc.declare_dram_parameter("xt", [IN1, (SY + 1) * SX * BL], bf16,
                                    isOutput=False)
    whh_d = nc.declare_dram_parameter("whht", [128, KC * MC * 128], bf16,
                                      isOutput=False)
    wp_d = nc.declare_dram_parameter("wpt", [128, KC * MC * 128], bf16,
                                     isOutput=False)
    wx_d = nc.declare_dram_parameter("wxt", [IN1, MC * 128], bf16,
                                     isOutput=False)
    out_d = nc.declare_dram_parameter("out", [128, KC, SY * SX, BL], bf16,
                                      isOutput=True)

    with tile.TileContext(nc) as tc:
        with (
            tc.tile_pool(name="persist", bufs=1) as persist,
            tc.tile_pool(name="gates", bufs=3) as gpool,
            tc.tile_pool(name="psum", bufs=1, space="PSUM") as pspool,
        ):
            whh_sb = persist.tile([128, KC, MC, 128], bf16)
            # one column per body step: written (with 0.0) by a dummy reader
            # of that step's pre-fill psum group, read as the (zero) bias of
            # the NEXT step's sigmoid(f).  This forces the scheduler to place
            # each pre-fill group inside its own step's tensor-engine gap
            # instead of piling all of them up at the half-row boundary.
            anchor = persist.tile([128, 2 * SX], f32)
            wp_sb = persist.tile([128, KC, MC, 128], bf16)
            wx_sb = persist.tile([IN1, MC, 128], bf16)
            c_sb = persist.tile([128, KC, BL], f32)
            hbfA = persist.tile([128, KC, SX, BL], bf16)
            hbfB = persist.tile([128, KC, SX, BL], bf16)
            xA = persist.tile([IN1, SX * BL], bf16)
            xB = persist.tile([IN1, SX * BL], bf16)
            xC = persist.tile([IN1, SX * BL], bf16)
            x0 = persist.tile([IN1, HS * BL], bf16)

            # two half-row PSUM gate regions, split per gate group (f/g/i/o,
            # one 2KB bank each) so the elementwise phase's dependencies are
            # per-group and each activation can start mid-burst.
            psA = tuple(pspool.tile([128, 4, HS, BL], f32, name=f"psA{g}")
                        for g in range(4))
            psB = tuple(pspool.tile([128, 4, HS, BL], f32, name=f"psB{g}")
                        for g in range(4))

            def slot_view(ps, s):
                """(tile, local slot) for global gate slot s."""
                return ps[s // 4], s % 4

            nc.sync.dma_start(out=whh_sb[:], in_=whh_d[:])
            nc.sync.dma_start(out=wp_sb[:], in_=wp_d[:])
            nc.sync.dma_start(out=wx_sb[:], in_=wx_d[:])
            nc.vector.memset(c_sb[:], 0.0)
            nc.vector.memset(hbfA[:], 0.0)
            nc.vector.memset(hbfB[:], 0.0)
            nc.vector.memset(anchor[:], 0.0)

            # pull the sigmoid/tanh ACT table load out of the loop
            warm = persist.tile([1, 1], f32)
            nc.vector.memset(warm[:], 0.0)
            nc.scalar.activation(out=warm[:], in_=warm[:], func=SIG)
            nc.scalar.activation(out=warm[:], in_=warm[:], func=TANH)

            # start=True clears the has_written bits for the WHOLE bank, so it
            # may only be issued on the first slot of each 4-slot bank; later
            # slots overwrite (bit cleared) then accumulate (bit set).
            def prefill(ps, s, xrow, xh, hprev, hh):
                """Accumulate slot s of half-row region ps with the x/bias and
                prev-row contributions: x half xh (0/1) of xrow, h half hh of
                hprev."""
                t, ls = slot_view(ps, s)
                nc.tensor.matmul(
                    t[:, ls, :, :], wx_sb[:, s, :],
                    xrow[:, xh * HS * BL:(xh + 1) * HS * BL],
                    start=(s % 4 == 0), stop=False)
                for k in range(KC):
                    nc.tensor.matmul(
                        t[:, ls, :, :], wp_sb[:, k, s, :],
                        hprev[:, k, hh * HS:(hh + 1) * HS, :],
                        start=False, stop=False)
                return t, ls

            # bootstrap: pre-fill region A with row 0 first half (prev row is
            # all zeros, so only the x/bias part).
            nc.sync.dma_start(out=x0[:], in_=x_d[:, 0:HS * BL])
            for s in range(MC):
                t, ls = slot_view(psA, s)
                nc.tensor.matmul(t[:, ls, :, :], wx_sb[:, s, :], x0[:],
                                 start=(s % 4 == 0), stop=False)

            def step(j, cur, prev, ps, scol, pre, gs):
                """One LSTM step: recurrence matmuls accumulate onto the four
                gate-group PSUM tiles at column scol, then the elementwise
                phase writes h (bf16) into cur[:, :, j, :].  pre() queues the
                next half-row's pre-fill matmuls; gs is this step's global
                index (0..63) in the 2-row body, used for the anchor chain."""
                for s in range(MC):
                    t, ls = slot_view(ps, s)
                    for k in range(KC):
                        rhs = (prev[:, k, SX - 1, :] if j == 0
                               else cur[:, k, j - 1, :])
                        nc.tensor.matmul(
                            t[:, ls, scol, :], whh_sb[:, k, s, :], rhs,
                            start=False, stop=(k == KC - 1))

                sf = gpool.tile([128, KC, BL], f32)
                tg = gpool.tile([128, KC, BL], f32)
                si = gpool.tile([128, KC, BL], f32)
                so = gpool.tile([128, KC, BL], f32)
                tc_t = gpool.tile([128, KC, BL], f32)
                fc = gpool.tile([128, KC, BL], f32)
                ig = gpool.tile([128, KC, BL], f32)
                pc = (gs - 1) % (2 * SX)
                nc.scalar.activation(out=sf[:], in_=ps[0][:, :, scol, :],
                                     func=SIG, bias=anchor[:, pc:pc + 1])
                nc.scalar.activation(out=tg[:], in_=ps[1][:, :, scol, :],
                                     func=TANH)
                nc.scalar.activation(out=si[:], in_=ps[2][:, :, scol, :],
                                     func=SIG)
                nc.vector.tensor_mul(fc[:], sf[:], c_sb[:])
                nc.vector.tensor_mul(ig[:], si[:], tg[:])
                nc.vector.tensor_add(c_sb[:], fc[:], ig[:])
                nc.scalar.activation(out=so[:], in_=ps[3][:, :, scol, :],
                                     func=SIG)
                nc.scalar.activation(out=tc_t[:], in_=c_sb[:], func=TANH)
                nc.vector.tensor_mul(cur[:, :, j, :], so[:], tc_t[:])

                tpre, lpre = pre()
                # anchor: 0.0 * (one element of the just-pre-filled group),
                # consumed as the zero bias of the NEXT step's sigmoid(f).
                # This makes the pre-fill a hard dependency of the next step,
                # pinning it into this step's tensor-engine idle gap.
                nc.vector.tensor_scalar_mul(
                    out=anchor[:, gs:gs + 1],
                    in0=tpre[:, lpre, 0, 0:1], scalar1=0.0)

            with tc.For_i(0, SY // 2) as iv:
                # x rows 2iv, 2iv+1, 2iv+2 (row 32 is zero padding)
                nc.gpsimd.dma_start(out=xA[:], in_=x_d[:, bass.ds(iv * 2 * SX * BL, SX * BL)])
                nc.gpsimd.dma_start(out=xB[:], in_=x_d[:, bass.ds((iv * 2 + 1) * SX * BL, SX * BL)])
                nc.gpsimd.dma_start(out=xC[:], in_=x_d[:, bass.ds((iv * 2 + 2) * SX * BL, SX * BL)])

                # ---- row 2iv: cur=hbfA, prev=hbfB
                for j in range(SX):
                    ps, scol = (psA, j) if j < HS else (psB, j - HS)
                    if j < HS:
                        # pre-fill psB slot j for this row's 2nd half
                        pre = lambda s=j: prefill(psB, s, xA, 1, hbfB, 1)
                    else:
                        # pre-fill psA slot j-16 for row 2iv+1 first half
                        pre = lambda s=j - HS: prefill(psA, s, xB, 0, hbfA, 0)
                    step(j, hbfA, hbfB, ps, scol, pre, j)
                nc.gpsimd.dma_start(
                    out=out_d[:, :, bass.ds(iv * 2 * SX, SX), :], in_=hbfA[:])

                # ---- row 2iv+1: cur=hbfB, prev=hbfA
                for j in range(SX):
                    ps, scol = (psA, j) if j < HS else (psB, j - HS)
                    if j < HS:
                        pre = lambda s=j: prefill(psB, s, xB, 1, hbfA, 1)
                    else:
                        # pre-fill psA for row 2iv+2 first half (x row 2iv+2;
                        # zero padding row at iv=15 - consumed never)
                        pre = lambda s=j - HS: prefill(psA, s, xC, 0, hbfB, 0)
                    step(j, hbfB, hbfA, ps, scol, pre, SX + j)
                nc.gpsimd.dma_start(
                    out=out_d[:, :, bass.ds((iv * 2 + 1) * SX, SX), :],
                    in_=hbfB[:])

    nc.compile()
    return nc


_CACHE = {}


def _get_module():
    if "m" not in _CACHE:
        _CACHE["m"] = _build_module()
    return _CACHE["m"]


def _prep_shared(W_ih, W_hh, b_ih, b_hh):
    perm = np.array(SLOT_TO_ORIG)
    wih_t = np.ascontiguousarray(W_ih.T.astype(np.float32))     # (560, 2048)
    bias = (b_ih + b_hh).astype(np.float32).reshape(MC, 128)[perm]
    wx = wih_t[:IN]                                             # (48, 2048)
    wx = wx.reshape(IN, MC, 128)[:, perm, :]
    wx = np.concatenate([wx, bias[None, :, :]], axis=0)         # (49, 16, 128)
    wx = wx.reshape(IN1, MC * 128)
    wp = wih_t[IN:]                                             # (512, 2048)
    wp = wp.reshape(KC, 128, MC, 128)[:, :, perm, :]
    wp = wp.transpose(1, 0, 2, 3).reshape(128, KC * MC * 128)
    whh = np.ascontiguousarray(W_hh.T.astype(np.float32))       # (512, 2048)
    whh = whh.reshape(KC, 128, MC, 128)[:, :, perm, :]
    whh = whh.transpose(1, 0, 2, 3).reshape(128, KC * MC * 128)
    return (wx.astype(BF16), wp.astype(BF16), whh.astype(BF16))


def _prep_x(batch):
    # xs[i, j, b, :] = patch (C,P,P) flattened, matching the reference
    xs = batch.reshape(B, C, SY, P, SX, P).transpose(2, 4, 0, 1, 3, 5)
    xs = xs.reshape(SY, SX, B, IN)
    per_core = []
    for c in range(NCORES):
        xc = xs[:, :, c * BL:(c + 1) * BL, :]          # (SY, SX, BL, IN)
        xc = xc.transpose(3, 0, 1, 2).reshape(IN, SY, SX * BL)
        xc = np.concatenate(
            [xc, np.ones((1, SY, SX * BL), np.float32)], axis=0)
        xc = np.concatenate(
            [xc, np.zeros((IN1, 1, SX * BL), np.float32)], axis=1)
        per_core.append(
            np.ascontiguousarray(xc.reshape(IN1, (SY + 1) * SX * BL))
            .astype(BF16))
    return per_core


def _run(batch, W_ih, W_hh, b_ih, b_hh, trace=False):
    from concourse.bass_utils import run_bass_kernel_spmd

    batch = np.asarray(batch, dtype=np.float32)
    wx, wp, whh = _prep_shared(
        np.asarray(W_ih), np.asarray(W_hh), np.asarray(b_ih), np.asarray(b_hh))
    xs = _prep_x(batch)

    nc = _get_module()
    in_maps = [
        {"xt": xs[c], "whht": whh, "wpt": wp, "wxt": wx}
        for c in range(NCORES)
    ]
    res = run_bass_kernel_spmd(nc, in_maps, list(range(NCORES)), trace=trace)

    outs = []
    for c in range(NCORES):
        arr = np.asarray(res.results[c]["out"]).astype(np.float32)
        # arr axes (128, KC, T, BL): reference's to_image is a raw reshape of
        # (B, T, NC) into (B, NC, SY, SX): flatten (BL, T, KC*128)->(BL, T*NC).
        arr = arr.transpose(3, 2, 1, 0).reshape(BL, NCELL, SY, SX)
        outs.append(arr)
    return np.concatenate(outs, axis=0), res


def kernel(batch, W_ih, W_hh, b_ih, b_hh):
    out, _ = _run(batch, W_ih, W_hh, b_ih, b_hh)
    return out
